# revision 14
# baseline (speedup 1.0000x reference)
"""Trainium2 Bass kernel for BaseSegHead (dynamic 1x1-conv seg logits).

Computes, for full inputs:
    qry_feats = in_feats @ qry_w.T + qry_b                  [1200, 32]
    key_map   = einsum('oc,bchw->bohw', key_w, feat_map) + key_b
    logits    = einsum('bnc,bchw->bnhw', qry_feats.reshape(4,300,32), key_map)
    out       = logits.reshape(1200, 160, 160)

Sharding: 8 cores = 4 batch images x 2 spatial (H) halves. Core c handles
batch b = c//2, rows h*80:(h+1)*80. Each core reads feat_map[b,:,rows,:],
its 300 queries, and writes its [300, 80*160] output shard -- no cross-core
communication and no duplicated feat_map reads.

This kernel is DMA-roofline bound (~6.55 MB feat in + 7.68 MB logits out
per core, fp16). Design notes:

* One HWDGE ring (Sync engine) carries every DMA in program order:
  [cpack, ft0, bpack, ft1..ft7, out0..out7]. 18 large transfers instead of
  ~40 small ones keeps the 16 SDMA engines backlogged; the issue stream
  (~0.7us per dma_start) never starves the packet stream.

* Transposed output layout. The main einsum is computed as
  outT[hw, query] = key_map[c, hw].T @ q[c, query]: key_map tiles are the
  matmul stationary operand, q the moving operand, so PSUM partitions are
  hw positions. Every SBUF partition then carries exactly 100 output rows
  of 600 B (vs 2-3 query rows of 25.6 KB in the query-major layout, which
  leaves a third of the DMA engines idle during output drain). Each block's
  store is one DMA with a fully contiguous per-partition destination
  (outT rows p*100+t). The host undoes the transpose (wall time, not HW).

* feat loads are [128, 2, w] (both 128-channel halves in one transfer, ~1MB).

* PE array tiling: the key projection (M=32) is 4-way column-tiled with the
  K-half loop OUTER, so the four column bands stream concurrently and the
  two accumulation halves of each band pipeline behind each other. The main
  einsum (K=32) is 4-way row-tiled with the m-tile loop OUTER and the band
  loop INNER: consecutive matmuls target different PE row-groups and run
  concurrently. Key quads are emitted one block ahead of the einsum so the
  PE never waits on the key-activation latency.

* PSUM: one uniform pool of four 2-bank tiles (all 8 banks). Einsum pairs
  (band 2P, 2P+1) share a tile, drained by one 600-column copy (amortizes
  the ~150-cycle DVE/ACT fixed cost); key quads and the q projection join
  the same rotation. Four buffers let the matmuls run ahead of the copy
  drain instead of in lockstep with it.

Precision: matmul operands fp16 (full PE rate, half DMA bytes); PSUM f32;
outputs rounded to fp16 for the store and upcast on host.
"""

import os
import sys

sys.path.insert(0, "/opt/trn_rl_repo")
os.environ.setdefault("MYCRO_LOCAL_CACHE", "1")

import numpy as np

BATCH = 4
N_PER = 300
IN_DIM = 256
KEY_DIM = 32
FH = FW = 160
HHALF = FH // 2            # 80 rows per core
HW = HHALF * FW            # 12800 spatial positions per core
N_CORES = 8

MMN = 512                  # one fp32 PSUM bank
# column blocks (multiples of 512). Small last block -> short drain tail.
BLOCK_W = (2048, 2048, 2048, 2048, 2048, 1536, 1024)
assert sum(BLOCK_W) == HW
N_BLOCKS = len(BLOCK_W)
BLOCK_C0 = tuple(sum(BLOCK_W[:k]) for k in range(N_BLOCKS))
N_TILES = HW // 128        # 100 hw-tiles of 128 positions
CPACK_W = 728              # fp16: qry_wT (64) + in_featsT (600) + key_wT (64)

_CACHE = {}


def build_nc():
    import concourse.bass as bass
    import concourse.bacc as bacc
    import concourse.mybir as mybir
    from concourse import tile

    f32 = mybir.dt.float32
    f16 = mybir.dt.float16
    Ident = mybir.ActivationFunctionType.Identity

    nc = bacc.Bacc("TRN2", target_bir_lowering=False, debug=False)

    featT = nc.dram_tensor("featT", [IN_DIM, HW], f16, kind="ExternalInput")
    cpack = nc.dram_tensor("cpack", [128, CPACK_W], f16, kind="ExternalInput")
    bpack = nc.dram_tensor("bpack", [128, 2], f32, kind="ExternalInput")
    out = nc.dram_tensor("out", [HW, N_PER], f16, kind="ExternalOutput")

    fv = featT.rearrange("(d p) c -> p d c", d=2)     # [128, 2, HW]
    ov = out.rearrange("(p t) n -> p t n", p=128)     # [128, N_TILES, N_PER]

    with tile.TileContext(nc) as tc:
        with (
            tc.tile_pool(name="const", bufs=1) as cpool,
            tc.tile_pool(name="fpool", bufs=N_BLOCKS) as fpool,
            tc.tile_pool(name="opool", bufs=4) as opool,
            tc.tile_pool(name="kmap", bufs=1) as kpool,
            tc.tile_pool(name="ps_main", bufs=4, space=bass.MemorySpace.PSUM) as ps_main,
        ):
            # --- input DMA stream (sync ring, program order) --------------
            ct = cpool.tile([128, CPACK_W], f16, name="ct")
            nc.sync.dma_start(ct[:], cpack[:])
            qw = (ct[:, 0:32], ct[:, 32:64])
            inT = (ct[:, 64:364], ct[:, 364:664])
            kw = (ct[:, 664:696], ct[:, 696:728])

            F = [None] * N_BLOCKS

            def load_block(k):
                col0, w = BLOCK_C0[k], BLOCK_W[k]
                ft = fpool.tile([128, 2, w], f16, name=f"feat_{k}", tag="fbf")
                nc.sync.dma_start(ft[:], fv[:, :, col0:col0 + w])
                F[k] = ft

            load_block(0)
            bt = cpool.tile([128, 2], f32, name="bt")
            nc.sync.dma_start(bt[:], bpack[:])
            qb = bt[:, 0:1]        # qry_b replicated in all four bands
            kb = bt[:, 1:2]        # key_b replicated in all four bands
            for k in range(1, N_BLOCKS):
                load_block(k)

            # --- drain load balancer: PSUM->SBUF ops go to whichever of
            # ACT/DVE has the least accumulated work (cost ~ cols + fixed).
            busy = {"v": 0, "s": 0}

            def drain(dst, src, bias=None):
                eng = "v" if busy["v"] <= busy["s"] else "s"
                n_cols = 1
                for dim in src.shape[1:]:
                    n_cols *= dim
                busy[eng] += n_cols + 151
                if bias is None:
                    if eng == "v":
                        nc.vector.tensor_copy(dst, src)
                    else:
                        nc.scalar.copy(dst, src)
                else:
                    if eng == "v":
                        nc.vector.tensor_scalar_add(dst, src, bias)
                    else:
                        nc.scalar.activation(dst, src, Ident, bias=bias)

            # --- qry projection, 4-way column-tiled (4 band copies) -------
            qpt = ps_main.tile([128, 2, MMN], f32, name="qpt", tag="mp")
            qp = qpt[:, 0, :]
            for d in range(2):
                for b in range(4):
                    nc.tensor.matmul(
                        qp[32 * b:32 * b + 32, 0:N_PER],
                        qw[d],
                        inT[d],
                        start=(d == 0),
                        stop=(d == 1),
                        tile_position=(0, 32 * b),
                    )
            q_sb = cpool.tile([128, N_PER], f16, name="q_sb")
            drain(q_sb[:], qp[:, 0:N_PER], bias=qb)

            # --- key_map: 4-way column-tiled, banded layout ---------------
            # subtile b of block k lives on SBUF partitions 32b..32b+31,
            # columns k*512..(k+1)*512; one PSUM bank holds a whole block.
            key_map = kpool.tile([128, N_BLOCKS * MMN], f16, name="key_map")

            def key_quad(k):
                nb = BLOCK_W[k] // MMN
                kpt = ps_main.tile([128, 2, MMN], f32, name=f"kp_{k}", tag="mp")
                kp = kpt[:, 0, :]
                for d in range(2):
                    for b in range(nb):
                        nc.tensor.matmul(
                            kp[32 * b:32 * b + 32, :],
                            kw[d],
                            F[k][:, d, b * MMN:(b + 1) * MMN],
                            start=(d == 0),
                            stop=(d == 1),
                            tile_position=(0, 32 * b),
                        )
                p = 32 * nb
                drain(
                    key_map[0:p, k * MMN:(k + 1) * MMN], kp[0:p, :],
                    bias=kb[0:p, :],
                )

            # --- per-block: einsum outT[hw, n] = key_map.T @ qT -----------
            # m-tile outer / band inner: consecutive matmuls hit different
            # PE row-groups and stream concurrently. Bands 2P,2P+1 share a
            # 2-bank PSUM tile; one copy drains both into the 4D ot tile
            # (dims: band, m-tile, query) whose flat order is hw-tile order.
            key_quad(0)
            for k in range(N_BLOCKS):
                if k + 1 < N_BLOCKS:
                    key_quad(k + 1)
                w = BLOCK_W[k]
                nb = w // MMN
                nt = 4 * nb
                ot = opool.tile([128, 4, 4, N_PER], f16, name=f"ot_{k}", tag="obuf")
                for m in range(4):
                    for P in range((nb + 1) // 2):
                        bb = [b for b in (2 * P, 2 * P + 1) if b < nb]
                        mp = ps_main.tile(
                            [128, 2, MMN], f32, name=f"mp_{k}_{m}_{P}", tag="mp"
                        )
                        for i, b in enumerate(bb):
                            nc.tensor.matmul(
                                mp[:, i, 0:N_PER],
                                key_map[
                                    32 * b:32 * b + 32,
                                    k * MMN + m * 128:k * MMN + (m + 1) * 128,
                                ],
                                q_sb[32 * b:32 * b + 32, :],
                                tile_position=(32 * b, 0),
                            )
                        drain(
                            ot[:, 2 * P:2 * P + len(bb), m, :],
                            mp[:, 0:len(bb), 0:N_PER],
                        )
                # store in band-contiguous halves: each half's destination
                # rows are consecutive and it issues as soon as its own four
                # copies land (one drain earlier than the full block).
                t0 = BLOCK_C0[k] // 128
                h = min(2, nb)
                nc.sync.dma_start(ov[:, t0:t0 + 4 * h, :], ot[:, 0:h, :, :])
                if nb > 2:
                    nc.sync.dma_start(
                        ov[:, t0 + 8:t0 + nt, :], ot[:, 2:nb, :, :]
                    )

    nc.compile()
    return nc


def _get_nc():
    if "nc" not in _CACHE:
        _CACHE["nc"] = build_nc()
    return _CACHE["nc"]


def make_in_maps(in_feats, feat_map, qry_w, qry_b, key_b, key_w):
    qwT = qry_w.T.astype(np.float16)                          # [256, 32]
    kwT = key_w.T.astype(np.float16)                          # [256, 32]
    bpack = np.zeros((128, 2), np.float32)
    bpack[:, 0] = np.tile(qry_b, 4)
    bpack[:, 1] = np.tile(key_b, 4)
    in_maps = []
    for c in range(N_CORES):
        b, h = divmod(c, 2)
        ifT = in_feats[b * N_PER:(b + 1) * N_PER].T.astype(np.float16)
        cpack = np.zeros((128, CPACK_W), np.float16)
        cpack[:, 0:32] = qwT[0:128]
        cpack[:, 32:64] = qwT[128:256]
        cpack[:, 64:364] = ifT[0:128]
        cpack[:, 364:664] = ifT[128:256]
        cpack[:, 664:696] = kwT[0:128]
        cpack[:, 696:728] = kwT[128:256]
        in_maps.append({
            "featT": np.ascontiguousarray(
                feat_map[b, :, h * HHALF:(h + 1) * HHALF, :]
            ).reshape(IN_DIM, HW).astype(np.float16),
            "cpack": cpack,
            "bpack": bpack,
        })
    return in_maps


def kernel(**inputs):
    in_feats = np.asarray(inputs["in_feats"], dtype=np.float32)
    feat_map = np.asarray(inputs["feat_map"], dtype=np.float32)
    qry_w = np.asarray(inputs["qry_w"], dtype=np.float32)
    qry_b = np.asarray(inputs["qry_b"], dtype=np.float32)
    key_w = np.asarray(inputs["key_w"], dtype=np.float32)
    key_b = np.asarray(inputs["key_b"], dtype=np.float32)

    from concourse import bass_utils

    nc = _get_nc()
    in_maps = make_in_maps(in_feats, feat_map, qry_w, qry_b, key_b, key_w)
    trace = os.environ.get("SEG_KERNEL_TRACE", "0") == "1"
    res = bass_utils.run_bass_kernel_spmd(
        nc, in_maps, core_ids=list(range(N_CORES)), trace=trace
    )
    _CACHE["last_result"] = res

    out = np.empty((BATCH * N_PER, FH, FW), dtype=np.float32)
    for c in range(N_CORES):
        b, h = divmod(c, 2)
        # device out row p*100 + t holds logits[:, t*128 + p]
        arr = res.results[c]["out"].reshape(128, N_TILES, N_PER)
        L = arr.transpose(2, 1, 0).reshape(N_PER, HW).astype(np.float32)
        out[b * N_PER:(b + 1) * N_PER, h * HHALF:(h + 1) * HHALF, :] = (
            L.reshape(N_PER, HHALF, FW)
        )
    return out
